# revision 13
# baseline (speedup 1.0000x reference)
"""Trainium2 Bass kernel for per-edge bilinear scoring (GNN message passing).

Reference computation:
    src, tgt = edge_label_index            # [E], [E]
    logits[e] = x_source[src[e]] @ W @ x_target[tgt[e]] + b

Strategy (8 NeuronCores, SPMD single program, per-core data):
  - Core c owns edges whose src is in rows [c*12500, (c+1)*12500).
  - SWDGE dma_gather throughput is ~2.1 ns per INDEX regardless of row
    bytes, so the optimization target is index count. Edges are bucketed
    by tgt into 32768-row windows (int16-addressable) and sorted by src
    within each bucket; runs of edges sharing a src are chunked into
    quads/pairs/singles. One ys index per CHUNK (not per edge) -- the DVE
    multiply broadcasts the gathered y row across the chunk's edge slots
    via a stride-0 access-pattern dim. xt stays one index per edge slot.
  - Block = 2048 edge slots, uniform chunk class R in {4,2,1} and uniform
    tgt bucket per block. The block schedule is static and shared across
    cores (SPMD); per-core overflow chunks demote (quad->2 pairs->2
    singles) and shortfalls pad with dummy chunks, dropped on output.
  - Phase 1 (device): y = x_source[chunk] @ W (12800 rows, bf16 out) to a
    DRAM scratch table; phase 2 gathers from it.
  - All gather tables are bf16 (256B rows). f32 accumulate in the reduce.
  - Host inverse-permutes the per-core slot outputs back to edge order.
"""

import contextlib
import os
import sys

for _p in ("/opt/trn_rl_repo",):
    if os.path.isdir(_p) and _p not in sys.path:
        sys.path.insert(0, _p)

import ml_dtypes
import numpy as np

import concourse.bacc as bacc
import concourse.bass as bass
import concourse.mybir as mybir
from concourse.bass_utils import run_bass_kernel_spmd
from concourse.library_config import mlp

N = 100000
E = 2000000
D = 128
NCORES = 8
CH = N // NCORES            # 12500 node rows per core chunk
CHP = 12800                 # chunk rows padded to 25 groups of 4*128
NI = 2048                   # edge slots per block (16 tiles)
NT = NI // 128              # 16
BUCK = 32768                # tgt bucket width (int16 window)
NB = 12                     # xt gather buffer rotation
NQ = 4                      # SWDGE queues

# static block schedule, shared by all cores: (bucket, R, nblocks)
# capacities sized for Poisson run stats of the 2M-edge list + slack.
SCHED = []
for _b in range(3):
    SCHED += [(_b, 8, 17), (_b, 4, 15), (_b, 2, 7), (_b, 1, 4)]
SCHED += [(3, 4, 1), (3, 2, 1), (3, 1, 2)]
NBLK = sum(n for _, _, n in SCHED)          # 133
CAP = NBLK * NI                             # 272384 edge slots
CAP_T = CAP // 128                          # 2128
# flat per-block (bucket, R) list + ys idx column offsets
BLOCKS = []
for _b, _r, _n in SCHED:
    BLOCKS += [(_b, _r)] * _n
YS_COLS = []                                # per-block ys idx column offset
_off = 0
for _b, _r in BLOCKS:
    YS_COLS.append(_off)
    _off += (NI // _r) // 16
YS_TOT_COLS = _off                          # 6496
# ys gather groups: one gather serves gs={4:4,2:2,1:1}[R] consecutive blocks
GROUPS = []                                 # (first_blk, nblocks, R)
_blk = 0
for _b, _r, _n in SCHED:
    _gs = {8: 16, 4: 8, 2: 4, 1: 2}[_r]
    _left = _n
    while _left:
        _g = min(_gs, _left)
        GROUPS.append((_blk, _g, _r))
        _blk += _g
        _left -= _g
GRP_OF = [None] * NBLK                      # blk -> (group idx, member m)
for _gi, (_fb, _gn, _r) in enumerate(GROUPS):
    for _m in range(_gn):
        GRP_OF[_fb + _m] = (_gi, _m)
NYB = 4                                     # ys gather buffer rotation
# xt gather groups: pair consecutive blocks within a schedule segment
XGROUPS = []                                # (first_blk, nblocks)
_blk = 0
for _b, _r, _n in SCHED:
    _left = _n
    while _left:
        _g = min(2, _left)
        XGROUPS.append((_blk, _g))
        _blk += _g
        _left -= _g
XG_OF = [None] * NBLK                       # blk -> (xgroup idx, member)
for _xi, (_fb, _gn) in enumerate(XGROUPS):
    for _m in range(_gn):
        XG_OF[_fb + _m] = (_xi, _m)
NXG = 6                                     # xt gather rotation (2 slots each)

F32 = mybir.dt.float32
BF16 = mybir.dt.bfloat16
I16 = mybir.dt.int16
NPBF16 = ml_dtypes.bfloat16

_NC_CACHE = None
_LAST_EXEC_NS = None
_LAST_RES = None
_TRACE = False


def _build_nc(num_devices=NCORES):
    n_xtiles = CHP // 128
    scols_xt = CAP // 16

    nc = bacc.Bacc("TRN2", target_bir_lowering=False, debug=False,
                   num_devices=num_devices, num_swdge_queues=NQ)
    xsT = nc.dram_tensor("xsT", [D, CHP], BF16, kind="ExternalInput")
    xt_tab = nc.dram_tensor("xt_tab", [N, D], BF16, kind="ExternalInput")
    W = nc.dram_tensor("W", [D, D], BF16, kind="ExternalInput")
    b_col = nc.dram_tensor("b_col", [D, 1], F32, kind="ExternalInput")
    src16 = nc.dram_tensor("src16", [128, YS_TOT_COLS], I16,
                           kind="ExternalInput")
    tgt16 = nc.dram_tensor("tgt16", [128, scols_xt], I16,
                           kind="ExternalInput")
    out = nc.dram_tensor("out", [128, CAP_T], F32, kind="ExternalOutput")
    y_tab = nc.dram_tensor("y_tab", [CHP, D], BF16, kind="ExternalOutput")

    with contextlib.ExitStack() as ctx:
        en = ctx.enter_context
        W_sb = en(nc.sbuf_tensor("W_sb", [D, D], BF16))
        b_sb = en(nc.sbuf_tensor("b_sb", [D, 1], F32))
        src_sb = en(nc.sbuf_tensor("src_sb", [128, YS_TOT_COLS], I16))
        tgt_sb = en(nc.sbuf_tensor("tgt_sb", [128, scols_xt], I16))
        logits = en(nc.sbuf_tensor("logits", [128, CAP_T], F32))
        xs_all = en(nc.sbuf_tensor("xs_all", [D, CHP], BF16))
        y_sb = [en(nc.sbuf_tensor(f"y_sb{i}", [128, 4, D], BF16))
                for i in range(2)]
        y_ps = [en(nc.psum_tensor(f"y_ps{i}", [128, D], F32)) for i in range(4)]
        ys_buf = [en(nc.sbuf_tensor(f"ys_buf{i}", [128, 2 * NT, D], BF16))
                  for i in range(NYB)]
        xt_all = en(nc.sbuf_tensor("xt_all", [128, 2 * NXG * NT, D], BF16))
        prod = en(nc.sbuf_tensor("prod", [128, NT, D], BF16))
        # semaphores
        ld = en(nc.semaphore("ld"))
        mm = en(nc.semaphore("mm"))
        cp = en(nc.semaphore("cp"))
        yst = [en(nc.semaphore(f"yst{i}")) for i in range(2)]
        g_ys = [en(nc.semaphore(f"gys{i}")) for i in range(NYB)]
        g_xt = [en(nc.semaphore(f"gxt{i}")) for i in range(NXG)]
        red = en(nc.semaphore("red"))
        bias = en(nc.semaphore("bias"))
        od = en(nc.semaphore("od"))

        ngrp = n_xtiles // 4

        with nc.Block() as block:

            @block.sync
            def _(sync):
                sync.dma_start(out=W_sb[:], in_=W[:]).then_inc(ld, 16)
                sync.dma_start(out=b_sb[:], in_=b_col[:]).then_inc(ld, 16)
                sync.dma_start(out=src_sb[:], in_=src16[:]).then_inc(ld, 16)
                sync.dma_start(out=tgt_sb[:], in_=tgt16[:]).then_inc(ld, 16)
                sync.dma_start(out=xs_all[:], in_=xsT[:]).then_inc(ld, 16)
                for j in range(ngrp):
                    sync.wait_ge(cp, 4 * (j + 1))
                    sync.dma_start(
                        out=y_tab[j * 512:(j + 1) * 512, :].rearrange(
                            "(g p) d -> p g d", p=128),
                        in_=y_sb[j % 2][:]).then_inc(yst[j % 2], 16)
                for qtr in range(4):
                    c0 = (NBLK * qtr // 4) * NT
                    c1 = (NBLK * (qtr + 1) // 4) * NT
                    sync.wait_ge(bias, qtr + 1)
                    sync.dma_start(out=out[:, c0:c1],
                                   in_=logits[:, c0:c1]).then_inc(od, 16)
                sync.wait_ge(od, 64)

            @block.tensor
            def _(tensor):
                tensor.wait_ge(ld, 80)
                for i in range(n_xtiles):
                    if i >= 4:
                        tensor.wait_ge(cp, i - 3)
                    tensor.matmul(
                        out=y_ps[i % 4][:],
                        lhsT=xs_all[:, i * 128:(i + 1) * 128],
                        rhs=W_sb[:], start=True,
                        stop=True).then_inc(mm, 1)

            @block.vector
            def _(vector):
                # phase 1: psum -> sbuf copies (f32 -> bf16)
                for i in range(n_xtiles):
                    j = i // 4
                    vector.wait_ge(mm, i + 1)
                    if j >= 2 and i % 4 == 0:
                        vector.wait_ge(yst[j % 2], 16 * (j // 2))
                    vector.tensor_copy(out=y_sb[j % 2][:, i % 4, :],
                                       in_=y_ps[i % 4][:]).then_inc(cp, 1)
                # phase 2: per-block broadcast-multiply + reduce
                for blk, (b, R) in enumerate(BLOCKS):
                    xi, xm = XG_OF[blk]
                    ks = (2 * (xi % NXG) + xm) * NT
                    gi, m = GRP_OF[blk]
                    kq = gi % NYB
                    nc_cols = 16 // R              # ys cols for this block
                    vector.wait_ge(g_ys[kq], 16 * (gi // NYB + 1))
                    vector.wait_ge(g_xt[xi % NXG], 16 * (xi // NXG + 1))
                    xt_sl = xt_all[:, ks:ks + NT, :]
                    c0 = m * nc_cols
                    if R == 1:
                        in0 = ys_buf[kq][:, c0:c0 + 16, :]
                        in1 = xt_sl
                        po = prod[:]
                    else:
                        in0 = (ys_buf[kq][:, c0:c0 + nc_cols, :]
                               .unsqueeze(2)
                               .broadcast_to([128, nc_cols, R, D]))
                        in1 = xt_sl.rearrange(
                            "p (c r) d -> p c r d", r=R)
                        po = prod[:].rearrange("p (c r) d -> p c r d", r=R)
                    vector.tensor_tensor(out=po, in0=in0, in1=in1,
                                         op=mybir.AluOpType.mult)
                    vector.tensor_reduce(
                        out=logits[:, blk * NT:(blk + 1) * NT],
                        in_=prod[:], axis=mybir.AxisListType.X,
                        op=mybir.AluOpType.add).then_inc(red, 1)
                for qtr in range(4):
                    b0 = NBLK * qtr // 4
                    b1 = NBLK * (qtr + 1) // 4
                    vector.wait_ge(red, b1)
                    vector.tensor_scalar_add(
                        out=logits[:, b0 * NT:b1 * NT],
                        in0=logits[:, b0 * NT:b1 * NT],
                        scalar1=b_sb[:, :1]).then_inc(bias, 1)

            @block.gpsimd
            def _(gpsimd):
                gpsimd.load_library(mlp)
                gpsimd.wait_ge(ld, 64)
                qctr = [0]

                def nextq():
                    q = qctr[0] % NQ
                    qctr[0] += 1
                    return q

                def gather_xt(xi):
                    fb, gn = XGROUPS[xi]
                    b = BLOCKS[fb][0]
                    kx = xi % NXG
                    nidx = gn * NI
                    c0 = fb * (NI // 16)
                    c1 = c0 + nidx // 16
                    hi = min(BUCK, N - b * BUCK)
                    if xi >= NXG:
                        pf, pn = XGROUPS[xi - NXG]
                        gpsimd.wait_ge(red, pf + pn)
                    gpsimd.dma_gather(
                        xt_all[:, 2 * kx * NT:2 * kx * NT + nidx // 128, :],
                        xt_tab[b * BUCK:b * BUCK + hi, :],
                        tgt_sb[:, c0:c1],
                        nidx, nidx, D, single_packet=False,
                        queue_num=nextq(),
                    ).then_inc(g_xt[kx], 16)

                def gather_ys(gi):
                    fb, gn, R = GROUPS[gi]
                    kq = gi % NYB
                    nidx = gn * (NI // R)
                    c0 = YS_COLS[fb]
                    c1 = c0 + nidx // 16
                    if gi >= NYB:
                        # slot reuse: all blocks of group gi-NYB reduced
                        pf, pn, _ = GROUPS[gi - NYB]
                        gpsimd.wait_ge(red, pf + pn)
                    gpsimd.dma_gather(
                        ys_buf[kq][:, :nidx // 128, :], y_tab[:, :],
                        src_sb[:, c0:c1],
                        nidx, nidx, D, single_packet=False,
                        queue_num=nextq(),
                    ).then_inc(g_ys[kq], 16)

                # prologue: xt gathers don't need y_tab
                for xi in range(NXG):
                    gather_xt(xi)
                gpsimd.wait_ge(yst[0], 16 * ((ngrp + 1) // 2))
                gpsimd.wait_ge(yst[1], 16 * (ngrp // 2))
                for blk, (b, R) in enumerate(BLOCKS):
                    gi, m = GRP_OF[blk]
                    if m == 0:
                        gather_ys(gi)
                    xi, xm = XG_OF[blk]
                    if xm == 0 and xi >= NXG:
                        gather_xt(xi)

    nc.compile()
    return nc


def _get_nc():
    global _NC_CACHE
    if _NC_CACHE is None:
        _NC_CACHE = _build_nc()
    return _NC_CACHE


def _wrap16(idx):
    """[n] int values -> [128, n//16] wrap: pos j -> (j%16, j//16), x8."""
    n = idx.size
    a = idx.astype(np.int16).reshape(n // 16, 16).T   # [16, n//16]
    return np.tile(a, (8, 1))


def _plan_core(src_l, tgt):
    """Chunk a core's edges. Returns {bucket: {R: [chunk arrays of edge
    positions (into the core edge list)]}}."""
    order = np.lexsort((src_l, tgt // BUCK))
    bid = tgt[order] // BUCK
    plan = {}
    for b in range(4):
        sel = order[bid == b]
        chunks = {8: [], 4: [], 2: [], 1: []}
        if sel.size:
            s = src_l[sel]
            starts = np.flatnonzero(np.r_[True, s[1:] != s[:-1]])
            ends = np.r_[starts[1:], s.size]
            for st, en in zip(starts, ends):
                run = sel[st:en]
                i = 0
                while en - st - i >= 8:
                    chunks[8].append(run[i:i + 8]); i += 8
                if en - st - i >= 4:
                    chunks[4].append(run[i:i + 4]); i += 4
                if en - st - i >= 2:
                    chunks[2].append(run[i:i + 2]); i += 2
                if en - st - i:
                    chunks[1].append(run[i:i + 1])
        plan[b] = chunks
    return plan


def _pack_core(src_l, tgt):
    """Build src16/tgt16 index tensors and the slot->edge map for one core.

    Returns (src16 [128, YS_TOT_COLS], tgt16 [128, CAP//16],
             slot_eid [NBLK, 128, NT] edge positions or -1).
    """
    plan = _plan_core(src_l, tgt)
    capn = {}
    for b, R, nb in SCHED:
        capn[(b, R)] = nb * (NI // R)
    for b in range(4):
        ch = plan[b]
        if (b, 8) not in capn:
            capn[(b, 8)] = 0
        if len(ch.get(8, [])) > capn[(b, 8)]:
            extra = ch[8][capn[(b, 8)]:]
            ch[8] = ch[8][:capn[(b, 8)]]
            for o in extra:
                ch[4] += [o[:4], o[4:]]
        if len(ch[4]) > capn[(b, 4)]:
            extra = ch[4][capn[(b, 4)]:]
            ch[4] = ch[4][:capn[(b, 4)]]
            for q in extra:
                ch[2] += [q[:2], q[2:]]
        if len(ch[2]) > capn[(b, 2)]:
            extra = ch[2][capn[(b, 2)]:]
            ch[2] = ch[2][:capn[(b, 2)]]
            for p in extra:
                ch[1] += [p[:1], p[1:]]
        assert len(ch[1]) <= capn[(b, 1)], \
            f"singles overflow b{b}: {len(ch[1])} > {capn[(b, 1)]}"
    ptr = {(b, R): 0 for b in range(4) for R in (8, 4, 2, 1)}
    ys_idx = np.zeros(YS_TOT_COLS * 16, dtype=np.int64)
    xt_idx = np.zeros(CAP, dtype=np.int64)
    slot_eid = np.full((NBLK, 128, NT), -1, dtype=np.int64)
    for blk, (b, R) in enumerate(BLOCKS):
        nch = NI // R
        lst = plan[b][R]
        p0 = ptr[(b, R)]
        cs = lst[p0:p0 + nch]
        ptr[(b, R)] = p0 + nch
        ysrow = np.zeros(nch, dtype=np.int64)
        tloc = np.zeros((128, NT), dtype=np.int64)   # [p, col]
        for j, chv in enumerate(cs):
            pj = j % 128
            cb = (j // 128) * R
            ysrow[j] = src_l[chv[0]]
            for r in range(len(chv)):
                slot_eid[blk, pj, cb + r] = chv[r]
                tloc[pj, cb + r] = tgt[chv[r]] - b * BUCK
        ys_idx[YS_COLS[blk] * 16:YS_COLS[blk] * 16 + nch] = ysrow
        # xt idx in position order i = col*128 + p
        xt_idx[blk * NI:(blk + 1) * NI] = tloc.T.reshape(-1)
    assert xt_idx.min() >= 0 and xt_idx.max() < BUCK
    assert ys_idx.min() >= 0 and ys_idx.max() < CH
    return _wrap16(ys_idx), _wrap16(xt_idx), slot_eid


def kernel(x_source, x_target, edge_label_index, W, b):
    global _LAST_EXEC_NS, _LAST_RES
    x_source = np.asarray(x_source, dtype=np.float32)
    x_target = np.asarray(x_target, dtype=np.float32)
    eli = np.asarray(edge_label_index)
    W = np.asarray(W, dtype=np.float32)
    bval = float(np.asarray(b))

    src = eli[0].astype(np.int64)
    tgt = eli[1].astype(np.int64)
    n_edges = src.shape[0]

    core_of = src // CH
    xt_bf = x_target.astype(NPBF16)
    W_bf = W.astype(NPBF16)
    b_colv = np.full((D, 1), bval, dtype=np.float32)

    in_maps = []
    slot_maps = []
    eidx_list = []
    for c in range(NCORES):
        eidx = np.flatnonzero(core_of == c)
        eidx_list.append(eidx)
        src_l = src[eidx] - c * CH
        src16, tgt16, slot_eid = _pack_core(src_l, tgt[eidx])
        slot_maps.append(slot_eid)
        xsT_c = np.zeros((D, CHP), dtype=np.float32)
        xsT_c[:, :CH] = x_source[c * CH:(c + 1) * CH].T
        in_maps.append({
            "xsT": xsT_c.astype(NPBF16),
            "xt_tab": xt_bf,
            "W": W_bf,
            "b_col": b_colv,
            "src16": np.ascontiguousarray(src16),
            "tgt16": np.ascontiguousarray(tgt16),
        })

    nc = _get_nc()
    res = run_bass_kernel_spmd(nc, in_maps, core_ids=list(range(NCORES)),
                               trace=_TRACE)
    _LAST_EXEC_NS = res.exec_time_ns
    _LAST_RES = res

    result = np.empty(n_edges, dtype=np.float32)
    for c in range(NCORES):
        out_c = res.results[c]["out"]            # [128, CAP_T]
        lg = np.asarray(out_c).reshape(128, NBLK, NT).transpose(1, 0, 2)
        sm = slot_maps[c]
        valid = sm >= 0
        result[eidx_list[c][sm[valid]]] = lg[valid]
    return result


# revision 15
# speedup vs baseline: 1.0444x; 1.0444x over previous
"""Trainium2 Bass kernel for per-edge bilinear scoring (GNN message passing).

Reference computation:
    src, tgt = edge_label_index            # [E], [E]
    logits[e] = x_source[src[e]] @ W @ x_target[tgt[e]] + b

Strategy (8 NeuronCores, SPMD single program, per-core data):
  - Core c owns edges whose src is in rows [c*12500, (c+1)*12500).
  - SWDGE dma_gather throughput is ~2.1 ns per INDEX regardless of row
    bytes, so the optimization target is index count. Edges are bucketed
    by tgt into 32768-row windows (int16-addressable) and sorted by src
    within each bucket; runs of edges sharing a src are chunked into
    quads/pairs/singles. One ys index per CHUNK (not per edge) -- the DVE
    multiply broadcasts the gathered y row across the chunk's edge slots
    via a stride-0 access-pattern dim. xt stays one index per edge slot.
  - Block = 2048 edge slots, uniform chunk class R in {4,2,1} and uniform
    tgt bucket per block. The block schedule is static and shared across
    cores (SPMD); per-core overflow chunks demote (quad->2 pairs->2
    singles) and shortfalls pad with dummy chunks, dropped on output.
  - Phase 1 (device): y = x_source[chunk] @ W (12800 rows, bf16 out) to a
    DRAM scratch table; phase 2 gathers from it.
  - All gather tables are bf16 (256B rows). f32 accumulate in the reduce.
  - Host inverse-permutes the per-core slot outputs back to edge order.
"""

import contextlib
import os
import sys

for _p in ("/opt/trn_rl_repo",):
    if os.path.isdir(_p) and _p not in sys.path:
        sys.path.insert(0, _p)

import ml_dtypes
import numpy as np

import concourse.bacc as bacc
import concourse.bass as bass
import concourse.mybir as mybir
from concourse.bass_utils import run_bass_kernel_spmd
from concourse.library_config import mlp

N = 100000
E = 2000000
D = 128
NCORES = 8
CH = N // NCORES            # 12500 node rows per core chunk
CHP = 12800                 # chunk rows padded to 25 groups of 4*128
NI = 2048                   # edge slots per block (16 tiles)
NT = NI // 128              # 16
BUCK = 32768                # tgt bucket width (int16 window)
NB = 14                     # xt gather buffer rotation
NQ = 4                      # SWDGE queues

# static block schedule, shared by all cores: (bucket, R, nblocks)
# capacities sized for Poisson run stats of the 2M-edge list + slack.
SCHED = []
for _b in range(3):
    SCHED += [(_b, 8, 17), (_b, 4, 15), (_b, 2, 7), (_b, 1, 4)]
SCHED += [(3, 4, 1), (3, 2, 1), (3, 1, 2)]
NBLK = sum(n for _, _, n in SCHED)          # 133
CAP = NBLK * NI                             # 272384 edge slots
CAP_T = CAP // 128                          # 2128
# flat per-block (bucket, R) list + ys idx column offsets
BLOCKS = []
for _b, _r, _n in SCHED:
    BLOCKS += [(_b, _r)] * _n
YS_COLS = []                                # per-block ys idx column offset
_off = 0
for _b, _r in BLOCKS:
    YS_COLS.append(_off)
    _off += (NI // _r) // 16
YS_TOT_COLS = _off                          # 6496
# ys gather groups: one gather serves gs={4:4,2:2,1:1}[R] consecutive blocks
GROUPS = []                                 # (first_blk, nblocks, R)
_blk = 0
for _b, _r, _n in SCHED:
    _gs = {8: 8, 4: 4, 2: 2, 1: 1}[_r]
    _left = _n
    while _left:
        _g = min(_gs, _left)
        GROUPS.append((_blk, _g, _r))
        _blk += _g
        _left -= _g
GRP_OF = [None] * NBLK                      # blk -> (group idx, member m)
for _gi, (_fb, _gn, _r) in enumerate(GROUPS):
    for _m in range(_gn):
        GRP_OF[_fb + _m] = (_gi, _m)
NYB = 8                                     # ys gather buffer rotation

F32 = mybir.dt.float32
BF16 = mybir.dt.bfloat16
I16 = mybir.dt.int16
NPBF16 = ml_dtypes.bfloat16

_NC_CACHE = None
_LAST_EXEC_NS = None
_LAST_RES = None
_TRACE = False


def _build_nc(num_devices=NCORES):
    n_xtiles = CHP // 128
    scols_xt = CAP // 16

    nc = bacc.Bacc("TRN2", target_bir_lowering=False, debug=False,
                   num_devices=num_devices, num_swdge_queues=NQ)
    xsT = nc.dram_tensor("xsT", [D, CHP], BF16, kind="ExternalInput")
    xt_tab = nc.dram_tensor("xt_tab", [N, D], BF16, kind="ExternalInput")
    W = nc.dram_tensor("W", [D, D], BF16, kind="ExternalInput")
    b_col = nc.dram_tensor("b_col", [D, 1], F32, kind="ExternalInput")
    src16 = nc.dram_tensor("src16", [128, YS_TOT_COLS], I16,
                           kind="ExternalInput")
    tgt16 = nc.dram_tensor("tgt16", [128, scols_xt], I16,
                           kind="ExternalInput")
    out = nc.dram_tensor("out", [128, CAP_T], F32, kind="ExternalOutput")
    y_tab = nc.dram_tensor("y_tab", [CHP, D], BF16, kind="ExternalOutput")

    with contextlib.ExitStack() as ctx:
        en = ctx.enter_context
        W_sb = en(nc.sbuf_tensor("W_sb", [D, D], BF16))
        b_sb = en(nc.sbuf_tensor("b_sb", [D, 1], F32))
        src_sb = en(nc.sbuf_tensor("src_sb", [128, YS_TOT_COLS], I16))
        tgt_sb = en(nc.sbuf_tensor("tgt_sb", [128, scols_xt], I16))
        logits = en(nc.sbuf_tensor("logits", [128, CAP_T], F32))
        xs_all = en(nc.sbuf_tensor("xs_all", [D, CHP], BF16))
        y_sb = [en(nc.sbuf_tensor(f"y_sb{i}", [128, 4, D], BF16))
                for i in range(2)]
        y_ps = [en(nc.psum_tensor(f"y_ps{i}", [128, D], F32)) for i in range(4)]
        ys_buf = [en(nc.sbuf_tensor(f"ys_buf{i}", [128, NT, D], BF16))
                  for i in range(NYB)]
        xt_buf = [en(nc.sbuf_tensor(f"xt_buf{i}", [128, NT, D], BF16))
                  for i in range(NB)]
        prod = en(nc.sbuf_tensor("prod", [128, NT, D], BF16))
        # semaphores
        ld = en(nc.semaphore("ld"))
        mm = en(nc.semaphore("mm"))
        cp = en(nc.semaphore("cp"))
        yst = [en(nc.semaphore(f"yst{i}")) for i in range(2)]
        g_ys = [en(nc.semaphore(f"gys{i}")) for i in range(NYB)]
        g_xt = [en(nc.semaphore(f"gxt{i}")) for i in range(NB)]
        red = en(nc.semaphore("red"))
        bias = en(nc.semaphore("bias"))
        od = en(nc.semaphore("od"))

        ngrp = n_xtiles // 4

        with nc.Block() as block:

            @block.sync
            def _(sync):
                sync.dma_start(out=tgt_sb[:], in_=tgt16[:]).then_inc(ld, 16)
                sync.dma_start(out=src_sb[:], in_=src16[:]).then_inc(ld, 16)
                sync.dma_start(out=W_sb[:], in_=W[:]).then_inc(ld, 16)
                sync.dma_start(out=b_sb[:], in_=b_col[:]).then_inc(ld, 16)
                sync.dma_start(out=xs_all[:], in_=xsT[:]).then_inc(ld, 16)
                for j in range(ngrp):
                    sync.wait_ge(cp, 4 * (j + 1))
                    sync.dma_start(
                        out=y_tab[j * 512:(j + 1) * 512, :].rearrange(
                            "(g p) d -> p g d", p=128),
                        in_=y_sb[j % 2][:]).then_inc(yst[j % 2], 16)
                for qtr in range(4):
                    c0 = (NBLK * qtr // 4) * NT
                    c1 = (NBLK * (qtr + 1) // 4) * NT
                    sync.wait_ge(bias, qtr + 1)
                    sync.dma_start(out=out[:, c0:c1],
                                   in_=logits[:, c0:c1]).then_inc(od, 16)
                sync.wait_ge(od, 64)

            @block.tensor
            def _(tensor):
                tensor.wait_ge(ld, 80)
                for i in range(n_xtiles):
                    if i >= 4:
                        tensor.wait_ge(cp, i - 3)
                    tensor.matmul(
                        out=y_ps[i % 4][:],
                        lhsT=xs_all[:, i * 128:(i + 1) * 128],
                        rhs=W_sb[:], start=True,
                        stop=True).then_inc(mm, 1)

            @block.vector
            def _(vector):
                # phase 1: psum -> sbuf copies (f32 -> bf16)
                for i in range(n_xtiles):
                    j = i // 4
                    vector.wait_ge(mm, i + 1)
                    if j >= 2 and i % 4 == 0:
                        vector.wait_ge(yst[j % 2], 16 * (j // 2))
                    vector.tensor_copy(out=y_sb[j % 2][:, i % 4, :],
                                       in_=y_ps[i % 4][:]).then_inc(cp, 1)
                # phase 2: per-block broadcast-multiply + reduce
                for blk, (b, R) in enumerate(BLOCKS):
                    k = blk % NB
                    gi, m = GRP_OF[blk]
                    kq = gi % NYB
                    nc_cols = 16 // R              # ys cols for this block
                    vector.wait_ge(g_ys[kq], 16 * (gi // NYB + 1))
                    vector.wait_ge(g_xt[k], 16 * (blk // NB + 1))
                    c0 = m * nc_cols
                    if R == 1:
                        in0 = ys_buf[kq][:, c0:c0 + 16, :]
                        in1 = xt_buf[k][:]
                        po = prod[:]
                    else:
                        in0 = (ys_buf[kq][:, c0:c0 + nc_cols, :]
                               .unsqueeze(2)
                               .broadcast_to([128, nc_cols, R, D]))
                        in1 = xt_buf[k][:].rearrange(
                            "p (c r) d -> p c r d", r=R)
                        po = prod[:].rearrange("p (c r) d -> p c r d", r=R)
                    vector.tensor_tensor(out=po, in0=in0, in1=in1,
                                         op=mybir.AluOpType.mult)
                    vector.tensor_reduce(
                        out=logits[:, blk * NT:(blk + 1) * NT],
                        in_=prod[:], axis=mybir.AxisListType.X,
                        op=mybir.AluOpType.add).then_inc(red, 1)
                vector.wait_ge(ld, 64)
                for qtr in range(4):
                    b0 = NBLK * qtr // 4
                    b1 = NBLK * (qtr + 1) // 4
                    vector.wait_ge(red, b1)
                    vector.tensor_scalar_add(
                        out=logits[:, b0 * NT:b1 * NT],
                        in0=logits[:, b0 * NT:b1 * NT],
                        scalar1=b_sb[:, :1]).then_inc(bias, 1)

            @block.gpsimd
            def _(gpsimd):
                gpsimd.load_library(mlp)
                gpsimd.wait_ge(ld, 16)
                qctr = [0]

                def nextq():
                    q = qctr[0] % NQ
                    qctr[0] += 1
                    return q

                def gather_xt(blk, b):
                    k = blk % NB
                    c0 = blk * (NI // 16)
                    c1 = (blk + 1) * (NI // 16)
                    hi = min(BUCK, N - b * BUCK)
                    gpsimd.dma_gather(
                        xt_buf[k][:], xt_tab[b * BUCK:b * BUCK + hi, :],
                        tgt_sb[:, c0:c1],
                        NI, NI, D, single_packet=False,
                        queue_num=nextq(),
                    ).then_inc(g_xt[k], 16)

                def gather_ys(gi):
                    fb, gn, R = GROUPS[gi]
                    kq = gi % NYB
                    nidx = gn * (NI // R)
                    c0 = YS_COLS[fb]
                    c1 = c0 + nidx // 16
                    if gi >= NYB:
                        # slot reuse: all blocks of group gi-NYB reduced
                        pf, pn, _ = GROUPS[gi - NYB]
                        gpsimd.wait_ge(red, pf + pn)
                    gpsimd.dma_gather(
                        ys_buf[kq][:, :nidx // 128, :], y_tab[:, :],
                        src_sb[:, c0:c1],
                        nidx, nidx, D, single_packet=False,
                        queue_num=nextq(),
                    ).then_inc(g_ys[kq], 16)

                # prologue: xt gathers don't need y_tab
                for blk in range(min(NB, NBLK)):
                    gather_xt(blk, BLOCKS[blk][0])
                gpsimd.wait_ge(ld, 32)
                gpsimd.wait_ge(yst[0], 16 * ((ngrp + 1) // 2))
                gpsimd.wait_ge(yst[1], 16 * (ngrp // 2))
                for blk, (b, R) in enumerate(BLOCKS):
                    gi, m = GRP_OF[blk]
                    if m == 0:
                        gather_ys(gi)
                    if blk >= NB:
                        gpsimd.wait_ge(red, blk - NB + 1)
                        gather_xt(blk, b)

    nc.compile()
    return nc


def _get_nc():
    global _NC_CACHE
    if _NC_CACHE is None:
        _NC_CACHE = _build_nc()
    return _NC_CACHE


def _wrap16(idx):
    """[n] int values -> [128, n//16] wrap: pos j -> (j%16, j//16), x8."""
    n = idx.size
    a = idx.astype(np.int16).reshape(n // 16, 16).T   # [16, n//16]
    return np.tile(a, (8, 1))


def _plan_core(src_l, tgt):
    """Chunk a core's edges. Returns {bucket: {R: [chunk arrays of edge
    positions (into the core edge list)]}}."""
    order = np.lexsort((src_l, tgt // BUCK))
    bid = tgt[order] // BUCK
    plan = {}
    for b in range(4):
        sel = order[bid == b]
        chunks = {8: [], 4: [], 2: [], 1: []}
        if sel.size:
            s = src_l[sel]
            starts = np.flatnonzero(np.r_[True, s[1:] != s[:-1]])
            ends = np.r_[starts[1:], s.size]
            for st, en in zip(starts, ends):
                run = sel[st:en]
                i = 0
                while en - st - i >= 8:
                    chunks[8].append(run[i:i + 8]); i += 8
                if en - st - i >= 4:
                    chunks[4].append(run[i:i + 4]); i += 4
                if en - st - i >= 2:
                    chunks[2].append(run[i:i + 2]); i += 2
                if en - st - i:
                    chunks[1].append(run[i:i + 1])
        plan[b] = chunks
    return plan


def _pack_core(src_l, tgt):
    """Build src16/tgt16 index tensors and the slot->edge map for one core.

    Returns (src16 [128, YS_TOT_COLS], tgt16 [128, CAP//16],
             slot_eid [NBLK, 128, NT] edge positions or -1).
    """
    plan = _plan_core(src_l, tgt)
    capn = {}
    for b, R, nb in SCHED:
        capn[(b, R)] = nb * (NI // R)
    for b in range(4):
        ch = plan[b]
        if (b, 8) not in capn:
            capn[(b, 8)] = 0
        if len(ch.get(8, [])) > capn[(b, 8)]:
            extra = ch[8][capn[(b, 8)]:]
            ch[8] = ch[8][:capn[(b, 8)]]
            for o in extra:
                ch[4] += [o[:4], o[4:]]
        if len(ch[4]) > capn[(b, 4)]:
            extra = ch[4][capn[(b, 4)]:]
            ch[4] = ch[4][:capn[(b, 4)]]
            for q in extra:
                ch[2] += [q[:2], q[2:]]
        if len(ch[2]) > capn[(b, 2)]:
            extra = ch[2][capn[(b, 2)]:]
            ch[2] = ch[2][:capn[(b, 2)]]
            for p in extra:
                ch[1] += [p[:1], p[1:]]
        assert len(ch[1]) <= capn[(b, 1)], \
            f"singles overflow b{b}: {len(ch[1])} > {capn[(b, 1)]}"
    ptr = {(b, R): 0 for b in range(4) for R in (8, 4, 2, 1)}
    ys_idx = np.zeros(YS_TOT_COLS * 16, dtype=np.int64)
    xt_idx = np.zeros(CAP, dtype=np.int64)
    slot_eid = np.full((NBLK, 128, NT), -1, dtype=np.int64)
    for blk, (b, R) in enumerate(BLOCKS):
        nch = NI // R
        lst = plan[b][R]
        p0 = ptr[(b, R)]
        cs = lst[p0:p0 + nch]
        ptr[(b, R)] = p0 + nch
        ysrow = np.zeros(nch, dtype=np.int64)
        tloc = np.zeros((128, NT), dtype=np.int64)   # [p, col]
        for j, chv in enumerate(cs):
            pj = j % 128
            cb = (j // 128) * R
            ysrow[j] = src_l[chv[0]]
            for r in range(len(chv)):
                slot_eid[blk, pj, cb + r] = chv[r]
                tloc[pj, cb + r] = tgt[chv[r]] - b * BUCK
        ys_idx[YS_COLS[blk] * 16:YS_COLS[blk] * 16 + nch] = ysrow
        # xt idx in position order i = col*128 + p
        xt_idx[blk * NI:(blk + 1) * NI] = tloc.T.reshape(-1)
    assert xt_idx.min() >= 0 and xt_idx.max() < BUCK
    assert ys_idx.min() >= 0 and ys_idx.max() < CH
    return _wrap16(ys_idx), _wrap16(xt_idx), slot_eid


def kernel(x_source, x_target, edge_label_index, W, b):
    global _LAST_EXEC_NS, _LAST_RES
    x_source = np.asarray(x_source, dtype=np.float32)
    x_target = np.asarray(x_target, dtype=np.float32)
    eli = np.asarray(edge_label_index)
    W = np.asarray(W, dtype=np.float32)
    bval = float(np.asarray(b))

    src = eli[0].astype(np.int64)
    tgt = eli[1].astype(np.int64)
    n_edges = src.shape[0]

    core_of = src // CH
    xt_bf = x_target.astype(NPBF16)
    W_bf = W.astype(NPBF16)
    b_colv = np.full((D, 1), bval, dtype=np.float32)

    in_maps = []
    slot_maps = []
    eidx_list = []
    for c in range(NCORES):
        eidx = np.flatnonzero(core_of == c)
        eidx_list.append(eidx)
        src_l = src[eidx] - c * CH
        src16, tgt16, slot_eid = _pack_core(src_l, tgt[eidx])
        slot_maps.append(slot_eid)
        xsT_c = np.zeros((D, CHP), dtype=np.float32)
        xsT_c[:, :CH] = x_source[c * CH:(c + 1) * CH].T
        in_maps.append({
            "xsT": xsT_c.astype(NPBF16),
            "xt_tab": xt_bf,
            "W": W_bf,
            "b_col": b_colv,
            "src16": np.ascontiguousarray(src16),
            "tgt16": np.ascontiguousarray(tgt16),
        })

    nc = _get_nc()
    res = run_bass_kernel_spmd(nc, in_maps, core_ids=list(range(NCORES)),
                               trace=_TRACE)
    _LAST_EXEC_NS = res.exec_time_ns
    _LAST_RES = res

    result = np.empty(n_edges, dtype=np.float32)
    for c in range(NCORES):
        out_c = res.results[c]["out"]            # [128, CAP_T]
        lg = np.asarray(out_c).reshape(128, NBLK, NT).transpose(1, 0, 2)
        sm = slot_maps[c]
        valid = sm >= 0
        result[eidx_list[c][sm[valid]]] = lg[valid]
    return result


# revision 16
# speedup vs baseline: 1.0524x; 1.0076x over previous
"""Trainium2 Bass kernel for per-edge bilinear scoring (GNN message passing).

Reference computation:
    src, tgt = edge_label_index            # [E], [E]
    logits[e] = x_source[src[e]] @ W @ x_target[tgt[e]] + b

Strategy (8 NeuronCores, SPMD single program, per-core data):
  - Core c owns edges whose src is in rows [c*12500, (c+1)*12500).
  - SWDGE dma_gather throughput is ~2.1 ns per INDEX regardless of row
    bytes, so the optimization target is index count. Edges are bucketed
    by tgt into 32768-row windows (int16-addressable) and sorted by src
    within each bucket; runs of edges sharing a src are chunked into
    quads/pairs/singles. One ys index per CHUNK (not per edge) -- the DVE
    multiply broadcasts the gathered y row across the chunk's edge slots
    via a stride-0 access-pattern dim. xt stays one index per edge slot.
  - Block = 2048 edge slots, uniform chunk class R in {4,2,1} and uniform
    tgt bucket per block. The block schedule is static and shared across
    cores (SPMD); per-core overflow chunks demote (quad->2 pairs->2
    singles) and shortfalls pad with dummy chunks, dropped on output.
  - Phase 1 (device): y = x_source[chunk] @ W (12800 rows, bf16 out) to a
    DRAM scratch table; phase 2 gathers from it.
  - All gather tables are bf16 (256B rows). f32 accumulate in the reduce.
  - Host inverse-permutes the per-core slot outputs back to edge order.
"""

import contextlib
import os
import sys

for _p in ("/opt/trn_rl_repo",):
    if os.path.isdir(_p) and _p not in sys.path:
        sys.path.insert(0, _p)

import ml_dtypes
import numpy as np

import concourse.bacc as bacc
import concourse.bass as bass
import concourse.mybir as mybir
from concourse.bass_utils import run_bass_kernel_spmd
from concourse.library_config import mlp

N = 100000
E = 2000000
D = 128
NCORES = 8
CH = N // NCORES            # 12500 node rows per core chunk
CHP = 12800                 # chunk rows padded to 25 groups of 4*128
NI = 2048                   # edge slots per block (16 tiles)
NT = NI // 128              # 16
BUCK = 32768                # tgt bucket width (int16 window)
NB = 12                     # xt gather buffer rotation
NQ = 4                      # SWDGE queues

# static block schedule, shared by all cores: (bucket, R, nblocks)
# capacities sized for Poisson run stats of the 2M-edge list + slack.
SCHED = []
for _b in range(3):
    SCHED += [(_b, 8, 17), (_b, 4, 15), (_b, 2, 7), (_b, 1, 4)]
SCHED += [(3, 4, 1), (3, 2, 1), (3, 1, 2)]
NBLK = sum(n for _, _, n in SCHED)          # 133
CAP = NBLK * NI                             # 272384 edge slots
CAP_T = CAP // 128                          # 2128
# flat per-block (bucket, R) list + ys idx column offsets
BLOCKS = []
for _b, _r, _n in SCHED:
    BLOCKS += [(_b, _r)] * _n
YS_COLS = []                                # per-block ys idx column offset
_off = 0
for _b, _r in BLOCKS:
    YS_COLS.append(_off)
    _off += (NI // _r) // 16
YS_TOT_COLS = _off                          # 6496
# ys gather groups: one gather serves gs={4:4,2:2,1:1}[R] consecutive blocks
GROUPS = []                                 # (first_blk, nblocks, R)
_blk = 0
for _b, _r, _n in SCHED:
    _gs = {8: 8, 4: 4, 2: 2, 1: 1}[_r]
    _left = _n
    while _left:
        _g = min(_gs, _left)
        GROUPS.append((_blk, _g, _r))
        _blk += _g
        _left -= _g
GRP_OF = [None] * NBLK                      # blk -> (group idx, member m)
for _gi, (_fb, _gn, _r) in enumerate(GROUPS):
    for _m in range(_gn):
        GRP_OF[_fb + _m] = (_gi, _m)
NYB = 6                                     # ys gather buffer rotation

F32 = mybir.dt.float32
BF16 = mybir.dt.bfloat16
I16 = mybir.dt.int16
NPBF16 = ml_dtypes.bfloat16

_NC_CACHE = None
_LAST_EXEC_NS = None
_LAST_RES = None
_TRACE = False


def _build_nc(num_devices=NCORES):
    n_xtiles = CHP // 128
    scols_xt = CAP // 16

    nc = bacc.Bacc("TRN2", target_bir_lowering=False, debug=False,
                   num_devices=num_devices, num_swdge_queues=NQ)
    xsT = nc.dram_tensor("xsT", [D, CHP], BF16, kind="ExternalInput")
    xt_tab = nc.dram_tensor("xt_tab", [N, D], BF16, kind="ExternalInput")
    W = nc.dram_tensor("W", [D, D], BF16, kind="ExternalInput")
    b_col = nc.dram_tensor("b_col", [D, 1], F32, kind="ExternalInput")
    src16 = nc.dram_tensor("src16", [128, YS_TOT_COLS], I16,
                           kind="ExternalInput")
    tgt16 = nc.dram_tensor("tgt16", [128, scols_xt], I16,
                           kind="ExternalInput")
    out = nc.dram_tensor("out", [128, CAP_T], F32, kind="ExternalOutput")
    y_tab = nc.dram_tensor("y_tab", [CHP, D], BF16, kind="ExternalOutput")

    with contextlib.ExitStack() as ctx:
        en = ctx.enter_context
        W_sb = en(nc.sbuf_tensor("W_sb", [D, D], BF16))
        b_sb = en(nc.sbuf_tensor("b_sb", [D, 1], F32))
        src_sb = en(nc.sbuf_tensor("src_sb", [128, YS_TOT_COLS], I16))
        tgt_sb = en(nc.sbuf_tensor("tgt_sb", [128, scols_xt], I16))
        logits = en(nc.sbuf_tensor("logits", [128, CAP_T], F32))
        xs_all = en(nc.sbuf_tensor("xs_all", [D, CHP], BF16))
        y_sb = [en(nc.sbuf_tensor(f"y_sb{i}", [128, 4, D], BF16))
                for i in range(2)]
        y_ps = [en(nc.psum_tensor(f"y_ps{i}", [128, D], F32)) for i in range(4)]
        ys_buf = [en(nc.sbuf_tensor(f"ys_buf{i}", [128, NT, D], BF16))
                  for i in range(NYB)]
        xt_buf = [en(nc.sbuf_tensor(f"xt_buf{i}", [128, NT, D], BF16))
                  for i in range(NB)]
        prod = en(nc.sbuf_tensor("prod", [128, NT, D], BF16))
        # semaphores
        ld = en(nc.semaphore("ld"))
        mm = en(nc.semaphore("mm"))
        cp = en(nc.semaphore("cp"))
        yst = [en(nc.semaphore(f"yst{i}")) for i in range(2)]
        g_ys = [en(nc.semaphore(f"gys{i}")) for i in range(NYB)]
        g_xt = [en(nc.semaphore(f"gxt{i}")) for i in range(NB)]
        red = en(nc.semaphore("red"))
        bias = en(nc.semaphore("bias"))
        od = en(nc.semaphore("od"))

        ngrp = n_xtiles // 4

        with nc.Block() as block:

            @block.sync
            def _(sync):
                sync.dma_start(out=W_sb[:], in_=W[:]).then_inc(ld, 16)
                sync.dma_start(out=b_sb[:], in_=b_col[:]).then_inc(ld, 16)
                sync.dma_start(out=src_sb[:], in_=src16[:]).then_inc(ld, 16)
                sync.dma_start(out=tgt_sb[:], in_=tgt16[:]).then_inc(ld, 16)
                sync.dma_start(out=xs_all[:], in_=xsT[:]).then_inc(ld, 16)
                for j in range(ngrp):
                    sync.wait_ge(cp, 4 * (j + 1))
                    sync.dma_start(
                        out=y_tab[j * 512:(j + 1) * 512, :].rearrange(
                            "(g p) d -> p g d", p=128),
                        in_=y_sb[j % 2][:]).then_inc(yst[j % 2], 16)
                for qtr in range(4):
                    c0 = (NBLK * qtr // 4) * NT
                    c1 = (NBLK * (qtr + 1) // 4) * NT
                    sync.wait_ge(bias, qtr + 1)
                    sync.dma_start(out=out[:, c0:c1],
                                   in_=logits[:, c0:c1]).then_inc(od, 16)
                sync.wait_ge(od, 64)

            @block.tensor
            def _(tensor):
                tensor.wait_ge(ld, 80)
                for i in range(n_xtiles):
                    if i >= 4:
                        tensor.wait_ge(cp, i - 3)
                    tensor.matmul(
                        out=y_ps[i % 4][:],
                        lhsT=xs_all[:, i * 128:(i + 1) * 128],
                        rhs=W_sb[:], start=True,
                        stop=True).then_inc(mm, 1)

            @block.vector
            def _(vector):
                # phase 1: psum -> sbuf copies (f32 -> bf16)
                for i in range(n_xtiles):
                    j = i // 4
                    vector.wait_ge(mm, i + 1)
                    if j >= 2 and i % 4 == 0:
                        vector.wait_ge(yst[j % 2], 16 * (j // 2))
                    vector.tensor_copy(out=y_sb[j % 2][:, i % 4, :],
                                       in_=y_ps[i % 4][:]).then_inc(cp, 1)
                # phase 2: per-block broadcast-multiply + reduce
                for blk, (b, R) in enumerate(BLOCKS):
                    k = blk % NB
                    gi, m = GRP_OF[blk]
                    kq = gi % NYB
                    nc_cols = 16 // R              # ys cols for this block
                    vector.wait_ge(g_ys[kq], 16 * (gi // NYB + 1))
                    vector.wait_ge(g_xt[k], 16 * (blk // NB + 1))
                    c0 = m * nc_cols
                    if R == 1:
                        in0 = ys_buf[kq][:, c0:c0 + 16, :]
                        in1 = xt_buf[k][:]
                        po = prod[:]
                    else:
                        in0 = (ys_buf[kq][:, c0:c0 + nc_cols, :]
                               .unsqueeze(2)
                               .broadcast_to([128, nc_cols, R, D]))
                        in1 = xt_buf[k][:].rearrange(
                            "p (c r) d -> p c r d", r=R)
                        po = prod[:].rearrange("p (c r) d -> p c r d", r=R)
                    vector.tensor_tensor(out=po, in0=in0, in1=in1,
                                         op=mybir.AluOpType.mult)
                    vector.tensor_reduce(
                        out=logits[:, blk * NT:(blk + 1) * NT],
                        in_=prod[:], axis=mybir.AxisListType.X,
                        op=mybir.AluOpType.add).then_inc(red, 1)
                for qtr in range(4):
                    b0 = NBLK * qtr // 4
                    b1 = NBLK * (qtr + 1) // 4
                    vector.wait_ge(red, b1)
                    vector.tensor_scalar_add(
                        out=logits[:, b0 * NT:b1 * NT],
                        in0=logits[:, b0 * NT:b1 * NT],
                        scalar1=b_sb[:, :1]).then_inc(bias, 1)

            @block.gpsimd
            def _(gpsimd):
                gpsimd.load_library(mlp)
                gpsimd.wait_ge(ld, 64)
                qctr = [0]

                def nextq():
                    q = qctr[0] % NQ
                    qctr[0] += 1
                    return q

                def gather_xt(blk, b):
                    k = blk % NB
                    c0 = blk * (NI // 16)
                    c1 = (blk + 1) * (NI // 16)
                    hi = min(BUCK, N - b * BUCK)
                    gpsimd.dma_gather(
                        xt_buf[k][:], xt_tab[b * BUCK:b * BUCK + hi, :],
                        tgt_sb[:, c0:c1],
                        NI, NI, D, single_packet=False,
                        queue_num=nextq(),
                    ).then_inc(g_xt[k], 16)

                def gather_ys(gi):
                    fb, gn, R = GROUPS[gi]
                    kq = gi % NYB
                    nidx = gn * (NI // R)
                    c0 = YS_COLS[fb]
                    c1 = c0 + nidx // 16
                    if gi >= NYB:
                        # slot reuse: all blocks of group gi-NYB reduced
                        pf, pn, _ = GROUPS[gi - NYB]
                        gpsimd.wait_ge(red, pf + pn)
                    gpsimd.dma_gather(
                        ys_buf[kq][:, :nidx // 128, :], y_tab[:, :],
                        src_sb[:, c0:c1],
                        nidx, nidx, D, single_packet=False,
                        queue_num=nextq(),
                    ).then_inc(g_ys[kq], 16)

                # prologue: xt gathers don't need y_tab
                for blk in range(min(NB, NBLK)):
                    gather_xt(blk, BLOCKS[blk][0])
                gpsimd.wait_ge(yst[0], 16 * ((ngrp + 1) // 2))
                gpsimd.wait_ge(yst[1], 16 * (ngrp // 2))
                for blk, (b, R) in enumerate(BLOCKS):
                    gi, m = GRP_OF[blk]
                    if m == 0:
                        gather_ys(gi)
                    if blk >= NB:
                        gpsimd.wait_ge(red, blk - NB + 1)
                        gather_xt(blk, b)

    nc.compile()
    return nc


def _get_nc():
    global _NC_CACHE
    if _NC_CACHE is None:
        _NC_CACHE = _build_nc()
    return _NC_CACHE


def _wrap16(idx):
    """[n] int values -> [128, n//16] wrap: pos j -> (j%16, j//16), x8."""
    n = idx.size
    a = idx.astype(np.int16).reshape(n // 16, 16).T   # [16, n//16]
    return np.tile(a, (8, 1))


def _plan_core(src_l, tgt):
    """Chunk a core's edges. Returns {bucket: {R: [chunk arrays of edge
    positions (into the core edge list)]}}."""
    order = np.lexsort((src_l, tgt // BUCK))
    bid = tgt[order] // BUCK
    plan = {}
    for b in range(4):
        sel = order[bid == b]
        chunks = {8: [], 4: [], 2: [], 1: []}
        if sel.size:
            s = src_l[sel]
            starts = np.flatnonzero(np.r_[True, s[1:] != s[:-1]])
            ends = np.r_[starts[1:], s.size]
            for st, en in zip(starts, ends):
                run = sel[st:en]
                i = 0
                while en - st - i >= 8:
                    chunks[8].append(run[i:i + 8]); i += 8
                if en - st - i >= 4:
                    chunks[4].append(run[i:i + 4]); i += 4
                if en - st - i >= 2:
                    chunks[2].append(run[i:i + 2]); i += 2
                if en - st - i:
                    chunks[1].append(run[i:i + 1])
        plan[b] = chunks
    return plan


def _pack_core(src_l, tgt):
    """Build src16/tgt16 index tensors and the slot->edge map for one core.

    Returns (src16 [128, YS_TOT_COLS], tgt16 [128, CAP//16],
             slot_eid [NBLK, 128, NT] edge positions or -1).
    """
    plan = _plan_core(src_l, tgt)
    capn = {}
    for b, R, nb in SCHED:
        capn[(b, R)] = nb * (NI // R)
    for b in range(4):
        ch = plan[b]
        if (b, 8) not in capn:
            capn[(b, 8)] = 0
        if len(ch.get(8, [])) > capn[(b, 8)]:
            extra = ch[8][capn[(b, 8)]:]
            ch[8] = ch[8][:capn[(b, 8)]]
            for o in extra:
                ch[4] += [o[:4], o[4:]]
        if len(ch[4]) > capn[(b, 4)]:
            extra = ch[4][capn[(b, 4)]:]
            ch[4] = ch[4][:capn[(b, 4)]]
            for q in extra:
                ch[2] += [q[:2], q[2:]]
        if len(ch[2]) > capn[(b, 2)]:
            extra = ch[2][capn[(b, 2)]:]
            ch[2] = ch[2][:capn[(b, 2)]]
            for p in extra:
                ch[1] += [p[:1], p[1:]]
        assert len(ch[1]) <= capn[(b, 1)], \
            f"singles overflow b{b}: {len(ch[1])} > {capn[(b, 1)]}"
    ptr = {(b, R): 0 for b in range(4) for R in (8, 4, 2, 1)}
    ys_idx = np.zeros(YS_TOT_COLS * 16, dtype=np.int64)
    xt_idx = np.zeros(CAP, dtype=np.int64)
    slot_eid = np.full((NBLK, 128, NT), -1, dtype=np.int64)
    for blk, (b, R) in enumerate(BLOCKS):
        nch = NI // R
        lst = plan[b][R]
        p0 = ptr[(b, R)]
        cs = lst[p0:p0 + nch]
        ptr[(b, R)] = p0 + nch
        ysrow = np.zeros(nch, dtype=np.int64)
        tloc = np.zeros((128, NT), dtype=np.int64)   # [p, col]
        for j, chv in enumerate(cs):
            pj = j % 128
            cb = (j // 128) * R
            ysrow[j] = src_l[chv[0]]
            for r in range(len(chv)):
                slot_eid[blk, pj, cb + r] = chv[r]
                tloc[pj, cb + r] = tgt[chv[r]] - b * BUCK
        ys_idx[YS_COLS[blk] * 16:YS_COLS[blk] * 16 + nch] = ysrow
        # xt idx in position order i = col*128 + p
        xt_idx[blk * NI:(blk + 1) * NI] = tloc.T.reshape(-1)
    assert xt_idx.min() >= 0 and xt_idx.max() < BUCK
    assert ys_idx.min() >= 0 and ys_idx.max() < CH
    return _wrap16(ys_idx), _wrap16(xt_idx), slot_eid


def kernel(x_source, x_target, edge_label_index, W, b):
    global _LAST_EXEC_NS, _LAST_RES
    x_source = np.asarray(x_source, dtype=np.float32)
    x_target = np.asarray(x_target, dtype=np.float32)
    eli = np.asarray(edge_label_index)
    W = np.asarray(W, dtype=np.float32)
    bval = float(np.asarray(b))

    src = eli[0].astype(np.int64)
    tgt = eli[1].astype(np.int64)
    n_edges = src.shape[0]

    core_of = src // CH
    xt_bf = x_target.astype(NPBF16)
    W_bf = W.astype(NPBF16)
    b_colv = np.full((D, 1), bval, dtype=np.float32)

    in_maps = []
    slot_maps = []
    eidx_list = []
    for c in range(NCORES):
        eidx = np.flatnonzero(core_of == c)
        eidx_list.append(eidx)
        src_l = src[eidx] - c * CH
        src16, tgt16, slot_eid = _pack_core(src_l, tgt[eidx])
        slot_maps.append(slot_eid)
        xsT_c = np.zeros((D, CHP), dtype=np.float32)
        xsT_c[:, :CH] = x_source[c * CH:(c + 1) * CH].T
        in_maps.append({
            "xsT": xsT_c.astype(NPBF16),
            "xt_tab": xt_bf,
            "W": W_bf,
            "b_col": b_colv,
            "src16": np.ascontiguousarray(src16),
            "tgt16": np.ascontiguousarray(tgt16),
        })

    nc = _get_nc()
    res = run_bass_kernel_spmd(nc, in_maps, core_ids=list(range(NCORES)),
                               trace=_TRACE)
    _LAST_EXEC_NS = res.exec_time_ns
    _LAST_RES = res

    result = np.empty(n_edges, dtype=np.float32)
    for c in range(NCORES):
        out_c = res.results[c]["out"]            # [128, CAP_T]
        lg = np.asarray(out_c).reshape(128, NBLK, NT).transpose(1, 0, 2)
        sm = slot_maps[c]
        valid = sm >= 0
        result[eidx_list[c][sm[valid]]] = lg[valid]
    return result
